# revision 53
# baseline (speedup 1.0000x reference)
"""Trainium2 Bass kernel for nn_Attention_65609920414302 (sparse multi-branch attention).

Sharding: 64 total heads (4 branches x 16 sub-heads) split as 8 heads per core
(core c = branch c//2, base-head half c%2). Each core computes Q/K/V projections
for its heads, RoPE, causal thresholded-softplus attention, and a partial W_O
matmul; the host sums the 8 partial outputs.

Math rescaling (S = pi/sqrt(3)):
  reference w_sig = w*sigmoid(S*w) with w = softplus(scores*m), thresholded at
  sink.  device   W = silu(S*w) = S*w_sig, thresholded at S*sink,
  probs = W / (sum_s W + S*(sink+1e-6)),  sink term = S*sink / (...).
The S factors cancel exactly.  softplus = ln(1 + exp(x)); exp and ln share the
natural_log_exp_and_others ACT table set; silu has its own set.

Structure (per core):
  - All inputs DMA'd in partition-major layouts (contiguous >=1KB lines).
  - K proj (PE) -> key_self (PE selector matmul) -> m = sqrt(recip/DH)
    (DVE+ACT) -> m broadcast via rank-1 matmul, folded into krope (DVE), so
    scores already include the per-key scale and exp needs no scale operand.
  - Causal mask = -60 constant matrix accumulated into diagonal score blocks
    via matmul (dies to 0 in fp16 exp).
  - ACT phase order: exp(p0) ln(p0) exp(p1) ln(p1) | silu h0..h3 | exp(p2)
    ln(p2) exp(p3) ln(p3) | silu h4..h7 -- 3 table loads on the hot path.
  - Per-query total comes free from a ones-row in the PV matmul; the sink bias
    is accumulated by a rank-1 matmul row, so DVE only does recip + normalize.
  - W_O runs per context quarter as soon as its pair's context is normalized,
    accumulating in fp32 SBUF.
"""

import math
import os
import numpy as np

D_MODEL = 1024
N_HEAD = 16
N_BR = 4
DH = 64
T = 1024
S = math.pi / math.sqrt(3.0)
N_CORES = 8
KT = 8           # C // 128 contraction tiles
L_LIST = [T - 128 * i for i in range(8)]
O_LIST = [sum(L_LIST[:i]) for i in range(8)]
W_COLS = sum(L_LIST)  # 4608

_NC_CACHE = [None]
LAST_RESULT = [None]  # stash for test harness (exec_time_ns etc.)


def _build_nc():
    import concourse.bass as bass
    from concourse import bacc
    import concourse.mybir as mybir
    import concourse.tile as tile
    from concourse.tile import add_dep_helper

    F32 = mybir.dt.float32
    F32R = mybir.dt.float32r
    F16 = mybir.dt.float16
    AF = mybir.ActivationFunctionType
    ALU = mybir.AluOpType

    nc = bacc.Bacc(None, target_bir_lowering=False, debug=False)

    # ---- DRAM parameters (per-core data; same program on all cores) ----
    # All tensors pre-rearranged on host so DMA lines are contiguous.
    XT = nc.declare_dram_parameter("XT", [128, KT, T], F16, isOutput=False)
    WQ = nc.declare_dram_parameter("WQ", [128, KT, 4, 128], F16, isOutput=False)
    WK = nc.declare_dram_parameter("WK", [128, KT, 4, 128], F16, isOutput=False)
    WV = nc.declare_dram_parameter("WV", [128, KT, 512], F16, isOutput=False)
    WO = nc.declare_dram_parameter("WO", [128, 4, 8, 128], F32R, isOutput=False)
    BQ = nc.declare_dram_parameter("BQ", [1, 512], F16, isOutput=False)
    BK = nc.declare_dram_parameter("BK", [1, 512], F16, isOutput=False)
    BV = nc.declare_dram_parameter("BV", [1, 512], F16, isOutput=False)
    COS = nc.declare_dram_parameter("COS", [128, T], F16, isOutput=False)
    SIN = nc.declare_dram_parameter("SIN", [128, T], F16, isOutput=False)
    PSW = nc.declare_dram_parameter("PSW", [128, 128], F16, isOutput=False)
    SEL = nc.declare_dram_parameter("SEL", [128, 4, 8], F16, isOutput=False)
    SELB = nc.declare_dram_parameter("SELB", [8, 8, 64], F16, isOutput=False)
    CMSK = nc.declare_dram_parameter("CMSK", [128, 128], F16, isOutput=False)
    IDF = nc.declare_dram_parameter("IDF", [128, 128], F16, isOutput=False)
    THR = nc.declare_dram_parameter("THR", [128, 8], F32, isOutput=False)
    VNS = nc.declare_dram_parameter("VNS", [64, 8], F32, isOutput=False)
    TBR = nc.declare_dram_parameter("TBR", [1, 8], F16, isOutput=False)
    ONES = nc.declare_dram_parameter("ONES", [1, 512], F16, isOutput=False)
    # One output slab per context-quarter pair; host sums them (with the
    # other cores), so no on-device y accumulation is needed.
    YT2 = nc.declare_dram_parameter("YT2", [2, D_MODEL, T], F32, isOutput=True)

    KDEBUG = bool(os.environ.get("KDEBUG"))
    if KDEBUG:
        DBG_M = nc.declare_dram_parameter("DBG_M", [8, T], F16, isOutput=True)
        DBG_KR = nc.declare_dram_parameter("DBG_KR", [128, T], F16, isOutput=True)
        DBG_QR = nc.declare_dram_parameter("DBG_QR", [128, T], F16, isOutput=True)
        DBG_WB = nc.declare_dram_parameter("DBG_WB", [128, 2, W_COLS], F16,
                                           isOutput=True)
        DBG_CX = nc.declare_dram_parameter("DBG_CX", [128, T], F32R, isOutput=True)
        DBG_QS = nc.declare_dram_parameter("DBG_QS", [128, T], F16, isOutput=True)
        DBG_T1 = nc.declare_dram_parameter("DBG_T1", [128, T], F16, isOutput=True)
        DBG_T2 = nc.declare_dram_parameter("DBG_T2", [128, T], F16, isOutput=True)
        DBG_SW = nc.declare_dram_parameter("DBG_SW", [128, T], F32, isOutput=True)
        DBG_C1 = nc.declare_dram_parameter("DBG_C1", [128, T], F32R, isOutput=True)
        DBG_C2 = nc.declare_dram_parameter("DBG_C2", [128, T], F32R, isOutput=True)
        DBG_C3 = nc.declare_dram_parameter("DBG_C3", [128, T], F32R, isOutput=True)
        DBG_VS = nc.declare_dram_parameter("DBG_VS", [128, 8, 8, 65], F16,
                                           isOutput=True)

    with tile.TileContext(nc) as tc:
        pc = tc.alloc_tile_pool(name="const", bufs=1)
        pp = tc.alloc_tile_pool(name="proj", bufs=1)
        pk = tc.alloc_tile_pool(name="keep", bufs=1)
        tr = tc.alloc_tile_pool(name="trans", bufs=2)
        pw = tc.alloc_tile_pool(name="wbuf", bufs=1)
        pj1 = tc.alloc_tile_pool(name="psum1", bufs=1, space="PSUM")
        cur_pj = [pj1]

        # ---- SBUF tiles ----
        cos_sb = pc.tile([128, T], F16)
        sin_sb = pc.tile([128, T], F16)
        psw_sb = pc.tile([128, 128], F16)
        sel_sb = pc.tile([128, 4, 8], F16)
        selb_sb = pc.tile([8, 8, 64], F16)
        cmsk_sb = pc.tile([128, 128], F16)
        idf_sb = pc.tile([128, 128], F16)
        thr_sb = pc.tile([128, 8], F32)
        vns_sb = pc.tile([64, 8], F32)
        tbr_sb = pc.tile([1, 8], F16)
        ones_r = pc.tile([1, 512], F16)
        m_sb = pc.tile([8, T], F32)
        m16 = pc.tile([8, T], F16)

        xt = pp.tile([128, KT, T], F16)
        wq = pp.tile([128, KT, 4, 128], F16)
        wk = pp.tile([128, KT, 4, 128], F16)
        wv = pp.tile([128, KT, 512], F16)
        bq = pp.tile([1, 512], F16)
        bk = pp.tile([1, 512], F16)
        bv = pp.tile([1, 512], F16)

        wo = pk.tile([128, 4, 8, 128], F32R)
        krope = [pk.tile([128, T], F16, name=f"krope{g}") for g in range(4)]
        qrope = [pk.tile([128, T], F16, name=f"qrope{g}") for g in range(4)]
        vstore = pk.tile([128, 8, 8, 65], F16)
        ctx = [pk.tile([128, T], F32R, name=f"ctx{g}") for g in range(4)]

        # ---- DMAs: small consts first (they land in ~1us), then the big
        # tensors one-shot each (long contiguous per-partition lines => few
        # descriptors). xt/wk gate the K projection.
        nc.sync.dma_start(out=ones_r, in_=ONES.ap())
        nc.sync.dma_start(out=psw_sb, in_=PSW.ap())
        nc.sync.dma_start(out=sel_sb, in_=SEL.ap())
        nc.sync.dma_start(out=selb_sb, in_=SELB.ap())
        nc.sync.dma_start(out=cmsk_sb, in_=CMSK.ap())
        nc.sync.dma_start(out=idf_sb, in_=IDF.ap())
        nc.sync.dma_start(out=bk, in_=BK.ap())
        nc.sync.dma_start(out=bq, in_=BQ.ap())
        nc.sync.dma_start(out=bv, in_=BV.ap())
        nc.sync.dma_start(out=thr_sb, in_=THR.ap())
        nc.sync.dma_start(out=vns_sb, in_=VNS.ap())
        nc.sync.dma_start(out=tbr_sb, in_=TBR.ap())
        nc.sync.dma_start(out=cos_sb, in_=COS.ap())
        nc.sync.dma_start(out=sin_sb, in_=SIN.ap())
        nc.sync.dma_start(out=wk, in_=WK.ap())
        nc.sync.dma_start(out=xt, in_=XT.ap())
        nc.sync.dma_start(out=wq, in_=WQ.ap())
        nc.sync.dma_start(out=wv, in_=WV.ap())
        nc.sync.dma_start(out=wo, in_=WO.ap())

        nc.vector.memset(vstore[:, :, :, 64:65], 1.0)

        def big():
            return cur_pj[0].tile([128, T], F32, tag="big", bufs=2,
                                  name="bigps")

        # warm up the PE clock (HAM) with dummy matmuls while DMAs stream in
        wu = big()
        for _ in range(24):
            nc.tensor.matmul(wu[0:1, 0:512], ones_r[0:1, 0:1], ones_r,
                             start=True, stop=True)

        # ---- projection + rope ----
        def proj_g(w_t, b_t, g):
            """Project group g (128 output dims) over all T; returns PSUM tile."""
            ps = big()
            for th in range(2):
                sl = slice(512 * th, 512 * (th + 1))
                for kt in range(KT):
                    nc.tensor.matmul(
                        ps[:, sl], w_t[:, kt, g, :], xt[:, kt, sl],
                        start=(kt == 0), stop=False,
                    )
                nc.tensor.matmul(
                    ps[:, sl], b_t[0:1, 128 * g:128 * (g + 1)], ones_r,
                    start=False, stop=True,
                )
            return ps

        def rope_g(qsb, g, out_t, dbg=False):
            """RoPE: out = qsb*cos + swap(qsb)*sin."""
            sw = big()
            for th in range(2):
                sl = slice(512 * th, 512 * (th + 1))
                nc.tensor.matmul(sw[:, sl], psw_sb, qsb[:, sl], start=True,
                                 stop=True)
            t1 = tr.tile([128, T], F16, tag="t1")
            nc.vector.tensor_tensor(t1, qsb, cos_sb, op=ALU.mult)
            t2 = tr.tile([128, T], F16, tag="t2")
            nc.vector.tensor_tensor(t2, sw, sin_sb, op=ALU.mult)
            if dbg:
                nc.sync.dma_start(out=DBG_QS.ap(), in_=qsb)
                nc.sync.dma_start(out=DBG_T1.ap(), in_=t1)
                nc.sync.dma_start(out=DBG_T2.ap(), in_=t2)
            # on DVE, not gpsimd: gpsimd tensor_tensor produced corrupt f16s
            # on partitions 64-127, and library thrash vs partition_broadcast
            # serialized the engine
            nc.vector.tensor_tensor(out_t, t1, t2, op=ALU.add)

        # K side: proj, key_self (accumulated via selector matmul), rope
        ks_ps = pj1.tile([8, T], F32, tag="ksps")
        for g in range(4):
            ps = proj_g(wk, bk, g)
            qsb = tr.tile([128, T], F16, tag="qsb")
            nc.scalar.copy(qsb, ps)          # ACT is idle in the prologue
            k2 = tr.tile([128, T], F16, tag="k2", bufs=1)
            nc.vector.tensor_tensor(k2, qsb, qsb, op=ALU.mult)
            for th in range(2):
                sl = slice(512 * th, 512 * (th + 1))
                nc.tensor.matmul(ks_ps[:, sl], sel_sb[:, g, :], k2[:, sl],
                                 start=(g == 0), stop=(g == 3))
            rope_g(qsb, g, krope[g])

        # m = sqrt(recip(max(ks,1e-6)) / DH) = ATTNSCALE / sqrt(key_self)
        nc.vector.tensor_scalar_max(m_sb, ks_ps, 1e-6)
        nc.vector.reciprocal_approx_fast(m_sb, m_sb)
        nc.scalar.activation(m16, m_sb, AF.Sqrt, scale=1.0 / DH)

        def fold_m(h):
            """krope[g][r0:r0+64, :] *= m[h, :] (broadcast via rank-1 matmul).

            The broadcast matmul always writes partitions 0:64 (base-64 PSUM
            writes poison later matmuls on recycled banks, and base-64 DVE
            PSUM reads silently read partition 0); a DVE copy stages it to the
            head's partition range in SBUF so the fold's two SBUF inputs share
            a base partition."""
            g, r0 = h // 2, 64 * (h % 2)
            mb = big()
            for th in range(2):
                sl = slice(512 * th, 512 * (th + 1))
                nc.tensor.matmul(mb[0:64, sl], selb_sb[:, h, :],
                                 m16[:, sl], start=True, stop=True)
            mbsb = tr.tile([128, T], F16, tag="mbsb", bufs=1)
            nc.vector.tensor_copy(mbsb[r0:r0 + 64, :], mb[0:64, :])
            nc.vector.tensor_tensor(
                krope[g][r0:r0 + 64, :], krope[g][r0:r0 + 64, :],
                mbsb[r0:r0 + 64, :], op=ALU.mult,
            )

        def qproj_rope(g):
            ps = proj_g(wq, bq, g)
            qsb = tr.tile([128, T], F16, tag="qsb")
            nc.scalar.copy(qsb, ps)  # see qp_rope: DVE copy corrupts here
            rope_g(qsb, g, qrope[g], dbg=(KDEBUG and g == 0))

        qproj_rope(0)
        fold_m(0)
        fold_m(1)

        # swap PSUM pools: prologue pool (incl. ksps) -> attention pool with
        # room for the double-buffered pair-packed score tiles
        pj1.release()
        pj2 = tc.alloc_tile_pool(name="psum2", bufs=1, space="PSUM")
        cur_pj[0] = pj2

        # V projection / Q projection emitted as small filler pieces so a
        # single filler never delays the next score block by more than ~1us
        vc_ps = {}

        def v_half(tt, half):
            """V projection for time block tt, contraction half `half`."""
            if half == 0:
                vc_ps[tt] = big()
            psv = vc_ps[tt]
            for kt in range(4 * half, 4 * half + 4):
                nc.tensor.matmul(
                    psv[:, 0:512], xt[:, kt, 128 * tt:128 * (tt + 1)],
                    wv[:, kt, :], start=(kt == 0), stop=False,
                )
            if half == 1:
                nc.tensor.matmul(psv[:, 0:512], ones_r[0:1, 0:128], bv,
                                 start=False, stop=True)
                nc.vector.tensor_copy(
                    vstore[:, tt, :, 0:64],
                    psv[:, 0:512].rearrange("p (h d) -> p h d", d=64),
                )

        qp_ps = {}

        def qp_mm(g, th, half):
            """Quarter of Q projection for group g."""
            if th == 0 and half == 0:
                qp_ps[g] = big()
            ps = qp_ps[g]
            sl = slice(512 * th, 512 * (th + 1))
            for kt in range(4 * half, 4 * half + 4):
                nc.tensor.matmul(ps[:, sl], wq[:, kt, g, :], xt[:, kt, sl],
                                 start=(kt == 0), stop=False)
            if half == 1:
                nc.tensor.matmul(
                    ps[:, sl], bq[0:1, 128 * g:128 * (g + 1)], ones_r,
                    start=False, stop=True,
                )

        def qp_rope(g):
            ps = qp_ps[g]
            qsb = tr.tile([128, T], F16, tag="qsb")
            # ACT copy, NOT vector.tensor_copy: the DVE fp32-PSUM -> fp16-SBUF
            # contiguous copy garbles even elements on partitions 64-127 here
            # (observed deterministically); the ACT path is clean.
            nc.scalar.copy(qsb, ps)
            rope_g(qsb, g, qrope[g])

        def pv_head(h, wbuf):
            """PV matmul (64 value dims); returns PSUM tile."""
            u = h % 2
            ps_pv = big()
            for i in range(8):
                t0 = 128 * i
                o = O_LIST[i]
                if t0 < 512:
                    chunks = [(t0, 512, 3), (512, T, 7)]
                else:
                    chunks = [(t0, T, 7)]
                for (a, b, last_i) in chunks:
                    nc.tensor.matmul(
                        ps_pv[0:64, a:b],
                        vstore[:, i, h, 0:64],
                        wbuf[:, u, o + (a - t0):o + (b - t0)],
                        start=(i == 0), stop=(i == last_i),
                    )
            return ps_pv

        def tot_head(h, wbuf):
            """Per-query total (sum over keys + sink bias) on PARTITION 0 of
            an sc-tag tile: keeps the reciprocal's PSUM read at base 0."""
            u = h % 2
            tps = cur_pj[0].tile([128, 2, 512], F32, tag="sc", bufs=2,
                                 name="totps")
            for i in range(8):
                t0 = 128 * i
                o = O_LIST[i]
                if t0 < 512:
                    chunks = [(t0, 512, 3), (512, T, 7)]
                else:
                    chunks = [(t0, T, 7)]
                for (a, b, last_i) in chunks:
                    cg, c0 = divmod(a, 512)
                    nc.tensor.matmul(
                        tps[0:1, cg, c0:c0 + (b - a)],
                        vstore[:, i, h, 64:65],
                        wbuf[:, u, o + (a - t0):o + (b - t0)],
                        start=(i == 0), stop=(i == last_i),
                    )
                if i == 0:
                    # sink bias accumulated into both column groups
                    for (a, b) in ((0, 512), (512, T)):
                        cg, c0 = divmod(a, 512)
                        nc.tensor.matmul(
                            tps[0:1, cg, c0:c0 + (b - a)],
                            tbr_sb[0:1, h:h + 1], ones_r[0:1, 0:b - a],
                            start=False, stop=False,
                        )
            return tps

        def norm_head(h, ps_pv, tps):
            """ctx[g][r0:r0+64, :] = (pv + S*sink*vnull) / total."""
            g, r0 = h // 2, 64 * (h % 2)
            tp = tr.tile([1, T], F32, tag="tp")
            nc.vector.reciprocal_approx_fast(tp, tps[0:1, :, :])
            gb = tr.tile([64, T], F32, tag="gb")
            nc.gpsimd.partition_broadcast(gb, tp, channels=64)
            nc.vector.scalar_tensor_tensor(
                out=ctx[g][r0:r0 + 64, :], in0=ps_pv[0:64, :],
                scalar=vns_sb[:, h:h + 1], in1=gb,
                op0=ALU.add, op1=ALU.mult,
            )

        def wo_pair(pr):
            """W_O partial for context quarters (2*pr, 2*pr+1) -> slab pr.

            The tail pair's PSUM->SBUF copies alternate DVE / ACT (ACT is idle
            once the last silu has issued)."""
            for mt in range(8):
                ps_o = big()
                for th in range(2):
                    sl = slice(512 * th, 512 * (th + 1))
                    for ci in range(2):
                        ct = 2 * pr + ci
                        nc.tensor.matmul(ps_o[:, sl], wo[:, ct, mt, :],
                                         ctx[ct][:, sl], start=(ci == 0),
                                         stop=(ci == 1))
                ysb = tr.tile([128, T], F32, tag="ysb")
                if pr == 1 and mt % 2 == 1:
                    nc.scalar.copy(ysb, ps_o)
                else:
                    nc.vector.tensor_copy(ysb, ps_o)
                nc.sync.dma_start(
                    out=YT2.ap()[pr, 128 * mt:128 * (mt + 1), :], in_=ysb
                )

        # ---- attention ----
        # ACT order: exp(p0) ln(p0) exp(p1) ln(p1) | silu h0..h3 |
        #            exp(p2) ln(p2) exp(p3) ln(p3) | silu h4..h7
        # PE fillers (Q proj, V proj, PV, W_O) are emitted between score blocks.
        wbuf_of = {}
        ln_of = {}
        prev_silu = []

        def scores_exp_pair(j, fillers):
            """Scores + exp for pair j (heads 2j, 2j+1); fillers: i -> [fn]."""
            exp_insts = []
            wbuf = pw.tile([128, 2, W_COLS], F16, tag="wbuf", bufs=3,
                           name=f"wbuf{j}")
            wbuf_of[j] = wbuf
            for i in range(8):
                t0 = 128 * i
                L = L_LIST[i]
                o = O_LIST[i]
                for c0 in range(0, L, 512):
                    cl = min(512, L - c0)
                    sct = cur_pj[0].tile([128, 2, 512], F32, tag="sc", bufs=2,
                                         name="scps")
                    for u in range(2):
                        r0 = 64 * u
                        nc.tensor.matmul(
                            sct[:, u, 0:cl],
                            krope[j][r0:r0 + 64, t0:t0 + 128],
                            qrope[j][r0:r0 + 64, t0 + c0:t0 + c0 + cl],
                            start=True, stop=(c0 > 0),
                        )
                        if c0 == 0:
                            # causal mask: -60 upper triangle of diag block
                            nc.tensor.matmul(
                                sct[:, u, 0:128], cmsk_sb, idf_sb,
                                start=False, stop=True,
                            )
                    e = nc.scalar.activation(
                        wbuf[:, :, o + c0:o + c0 + cl], sct[:, :, 0:cl], AF.Exp
                    )
                    for si in prev_silu:
                        add_dep_helper(e.ins, si.ins, sync=False,
                                       reason="act table phase order")
                    exp_insts.append(e)
                for fn in fillers.get(i, []):
                    fn()
            ln = nc.scalar.activation(wbuf[:, :, :], wbuf[:, :, :], AF.Ln,
                                      bias=1.0)
            for e in exp_insts:
                add_dep_helper(ln.ins, e.ins, sync=False,
                               reason="act table phase order")
            ln_of[j] = ln

        def silu_head(h, extra_deps):
            j, u = h // 2, h % 2
            si = nc.scalar.activation(
                wbuf_of[j][:, u, :], wbuf_of[j][:, u, :], AF.Silu, scale=S
            )
            for d in extra_deps:
                add_dep_helper(si.ins, d.ins, sync=False,
                               reason="act table phase order")
            return si

        def thr_head(h):
            j, u = h // 2, h % 2
            wbuf = wbuf_of[j]
            nc.vector.scalar_tensor_tensor(
                out=wbuf[:, u, :], in0=wbuf[:, u, :],
                scalar=thr_sb[:, h:h + 1], in1=wbuf[:, u, :],
                op0=ALU.is_ge, op1=ALU.mult,
            )

        # pair 0: fillers = fold m for remaining heads + Q proj g1 quarters
        scores_exp_pair(0, {
            0: [lambda: fold_m(2), lambda: fold_m(3)],
            1: [lambda: fold_m(4), lambda: fold_m(5)],
            2: [lambda: fold_m(6), lambda: fold_m(7)],
            3: [lambda: qp_mm(1, 0, 0)],
            4: [lambda: qp_mm(1, 0, 1)],
            5: [lambda: qp_mm(1, 1, 0)],
            6: [lambda: qp_mm(1, 1, 1)],
            7: [lambda: qp_rope(1)],
        })
        # pair 1: V chunks 0..3
        scores_exp_pair(1, {
            0: [lambda: v_half(0, 0)],
            1: [lambda: v_half(0, 1)],
            2: [lambda: v_half(1, 0)],
            3: [lambda: v_half(1, 1)],
            4: [lambda: v_half(2, 0)],
            5: [lambda: v_half(2, 1)],
            6: [lambda: v_half(3, 0)],
            7: [lambda: v_half(3, 1)],
        })

        # silu section A: heads 0..3, with thr/PV/norm chasing; V chunks 4..7
        # and Q proj g2 fill the PE while ACT does the silus
        silu_A = []
        sectA_pe = [
            [lambda: v_half(4, 0), lambda: v_half(4, 1),
             lambda: v_half(5, 0), lambda: v_half(5, 1),
             lambda: v_half(6, 0), lambda: v_half(6, 1),
             lambda: v_half(7, 0), lambda: v_half(7, 1)],
            [lambda: qp_mm(2, 0, 0), lambda: qp_mm(2, 0, 1)],
            [lambda: qp_mm(2, 1, 0), lambda: qp_mm(2, 1, 1)],
            [lambda: qp_rope(2)],
        ]
        for h in (0, 1, 2, 3):
            silu_A.append(silu_head(h, [ln_of[0], ln_of[1]]))
            thr_head(h)
            for fn in sectA_pe[h]:
                fn()
            tps = tot_head(h, wbuf_of[h // 2])
            ps_pv = pv_head(h, wbuf_of[h // 2])
            norm_head(h, ps_pv, tps)
        wo_pair(0)
        prev_silu = silu_A

        scores_exp_pair(2, {
            0: [lambda: qp_mm(3, 0, 0)],
            1: [lambda: qp_mm(3, 0, 1)],
            2: [lambda: qp_mm(3, 1, 0)],
            3: [lambda: qp_mm(3, 1, 1)],
            4: [lambda: qp_rope(3)],
        })
        scores_exp_pair(3, {})

        # silu section B: heads 4..7, thr/PV/norm/WO(ct2, ct3) + writeback
        for h in (4, 5, 6, 7):
            silu_head(h, [ln_of[2], ln_of[3]])
            thr_head(h)
            tps = tot_head(h, wbuf_of[h // 2])
            ps_pv = pv_head(h, wbuf_of[h // 2])
            norm_head(h, ps_pv, tps)
        wo_pair(1)

        if KDEBUG:
            nc.sync.dma_start(out=DBG_M.ap(), in_=m16)
            nc.sync.dma_start(out=DBG_KR.ap(), in_=krope[0])
            nc.sync.dma_start(out=DBG_QR.ap(), in_=qrope[0])
            # wbuf bufs=3: only pair 3's buffer is safely live at the end
            nc.sync.dma_start(out=DBG_WB.ap(), in_=wbuf_of[3])
            nc.sync.dma_start(out=DBG_CX.ap(), in_=ctx[0])
            nc.sync.dma_start(out=DBG_C1.ap(), in_=ctx[1])
            nc.sync.dma_start(out=DBG_C2.ap(), in_=ctx[2])
            nc.sync.dma_start(out=DBG_C3.ap(), in_=ctx[3])
            nc.sync.dma_start(out=DBG_VS.ap(), in_=vstore)

        pj2.release()
        pw.release()
        tr.release()
        pk.release()
        pp.release()
        pc.release()

    # Route exp and ln to the combined natural_log_exp_and_others ACT table
    # set (one load covers both phases): strip those functions from the
    # earlier-indexed single-function sets so the set picker can't choose
    # them. Indices (= act_func_set_id walrus remaps by) stay intact.
    import concourse.bacc as _bacc_mod
    from concourse.hw_specs import get_activation_tables as _gat

    def _gat_patched(arch):
        t = {k: set(v) for k, v in _gat(arch).items()}
        if "natural_log_exp_and_others" in t:
            for k in t:
                if k != "natural_log_exp_and_others":
                    t[k].discard(AF.Exp)
                    t[k].discard(AF.Ln)
        return t

    _bacc_mod.get_activation_tables = _gat_patched
    try:
        nc.finalize()
    finally:
        _bacc_mod.get_activation_tables = _gat
    return nc


def _host_inputs(inputs):
    """Build the 8 per-core input maps from full inputs."""
    X = np.asarray(inputs["X"], dtype=np.float32)
    W_Q = np.asarray(inputs["W_Q"], dtype=np.float32)
    b_Q = np.asarray(inputs["b_Q"], dtype=np.float32)
    W_K = np.asarray(inputs["W_K"], dtype=np.float32)
    b_K = np.asarray(inputs["b_K"], dtype=np.float32)
    W_V = np.asarray(inputs["W_V"], dtype=np.float32)
    b_V = np.asarray(inputs["b_V"], dtype=np.float32)
    sink = np.asarray(inputs["sink_scalars"], dtype=np.float32)
    v_nulls = np.asarray(inputs["v_nulls"], dtype=np.float32)
    W_O = np.asarray(inputs["W_O"], dtype=np.float32)

    XT = np.ascontiguousarray(
        X[0].T.reshape(KT, 128, T).transpose(1, 0, 2)
    ).astype(np.float16)  # [128, kt, T]

    # channel permutation (evens then odds) within each head's 64 channels
    perm64 = np.concatenate([np.arange(0, 64, 2), np.arange(1, 64, 2)])
    perm512 = (np.arange(8)[:, None] * 64 + perm64[None, :]).reshape(-1)

    def wqk_layout(w):  # [1024, 512] -> [128, kt, 4, 128]
        return np.ascontiguousarray(
            w.reshape(KT, 128, 4, 128).transpose(1, 0, 2, 3)
        ).astype(np.float16)

    # RoPE tables, matching reference float32 math
    invf = (1.0 / (10000.0 ** (np.arange(0, DH, 2, dtype=np.float32) / DH))).astype(
        np.float32
    )
    freqs = np.arange(T, dtype=np.float32)[:, None] * invf[None, :]  # [T, 32]
    cos32 = np.cos(freqs).T  # [32, T]
    sin32 = np.sin(freqs).T
    cos128 = np.tile(cos32, (4, 1)).astype(np.float16)
    sin128 = np.concatenate([-sin32, sin32, -sin32, sin32], axis=0).astype(np.float16)

    # swap matrix: out[p] = q[partner(p)]; lhsT[p', p] = 1 iff p' = partner(p)
    pswap = np.zeros((128, 128), dtype=np.float16)
    for p in range(128):
        partner = p + 32 if (p % 64) < 32 else p - 32
        pswap[partner, p] = 1.0

    # key_self selectors: sel[g][p, h] = 1 iff h == 2g + (p >= 64)
    sel = np.zeros((128, 4, 8), dtype=np.float16)
    for g in range(4):
        sel[0:64, g, 2 * g] = 1.0
        sel[64:128, g, 2 * g + 1] = 1.0

    # m broadcast selector: selb[r, h, :] = 1 iff r == h
    selb = np.zeros((8, 8, 64), dtype=np.float16)
    for h in range(8):
        selb[h, h, :] = 1.0

    # causal mask matrix: out[p, c] += cmsk[c, p]; want -60 where c < p
    cmsk = np.zeros((128, 128), dtype=np.float16)
    for r in range(128):
        cmsk[r, r + 1:] = -60.0
    idf = np.eye(128, dtype=np.float16)

    in_maps = []
    for c in range(N_CORES):
        n, half = c // 2, c % 2
        qs = slice(512 * c, 512 * (c + 1))
        ks = slice(512 * half, 512 * (half + 1))
        heads = np.arange(8 * c, 8 * c + 8)
        sinks = sink[heads]  # [8]
        thr = np.tile((S * sinks).astype(np.float32)[None, :], (128, 1))
        tbr = (S * (sinks + 1e-6)).astype(np.float16)[None, :]
        vn = v_nulls[n].reshape(N_HEAD, DH)  # base-head x d
        vns = np.zeros((64, 8), dtype=np.float32)
        for h in range(8):
            bh = (8 * half) + h  # base head index within branch
            vns[:, h] = S * sinks[h] * vn[bh]
        wo = np.ascontiguousarray(
            (0.25 * W_O[n, ks, :]).reshape(4, 128, 8, 128).transpose(1, 0, 2, 3)
        ).astype(np.float32)
        in_maps.append(
            {
                "XT": XT,
                "WQ": wqk_layout(W_Q[:, qs][:, perm512]),
                "BQ": np.ascontiguousarray(b_Q[qs][perm512])[None, :].astype(np.float16),
                "WK": wqk_layout(W_K[:, ks][:, perm512]),
                "BK": np.ascontiguousarray(b_K[ks][perm512])[None, :].astype(np.float16),
                "WV": np.ascontiguousarray(
                    W_V[:, ks].reshape(KT, 128, 512).transpose(1, 0, 2)
                ).astype(np.float16),
                "BV": np.ascontiguousarray(b_V[ks])[None, :].astype(np.float16),
                "WO": wo,
                "COS": cos128,
                "SIN": sin128,
                "PSW": pswap,
                "SEL": sel,
                "SELB": selb,
                "CMSK": cmsk,
                "IDF": idf,
                "THR": thr,
                "VNS": vns,
                "TBR": tbr,
                "ONES": np.ones((1, 512), dtype=np.float16),
            }
        )
    return in_maps


def kernel(**inputs) -> np.ndarray:
    from concourse.bass_utils import run_bass_kernel_spmd

    in_maps = _host_inputs(inputs)
    if _NC_CACHE[0] is None:
        _NC_CACHE[0] = _build_nc()
    nc = _NC_CACHE[0]
    trace = bool(os.environ.get("KBENCH_TRACE"))
    res = run_bass_kernel_spmd(
        nc, in_maps, core_ids=list(range(N_CORES)), trace=trace
    )
    LAST_RESULT[0] = res
    if trace and res.exec_time_ns is not None:
        print(f"HW exec time: {res.exec_time_ns} ns")

    W_O_bias = np.asarray(inputs["W_O_bias"], dtype=np.float32)
    y = np.zeros((D_MODEL, T), dtype=np.float32)
    for r in res.results:
        y += r["YT2"].reshape(2, D_MODEL, T).sum(axis=0)
    y = y.T + W_O_bias.mean(axis=0)[None, :]
    return y[None, :, :].astype(np.float32)
